# revision 1
# baseline (speedup 1.0000x reference)
"""SENSE conjugate-gradient MRI reconstruction on 8 Trainium2 NeuronCores.

Batch-parallel: each of the 8 cores solves one batch element's 10-iteration CG
where AtA(p) = sum_c conj(csm_c) * ifft2(mask * fft2(csm_c * p)) + lam * p.

The 320-point 2D FFTs are computed as dense DFT matmuls on the tensor engine
in fp32r (FP22 products, FP32 accumulate). Using the symmetric DFT matrix F
and the engine primitive mm(L, R) = L^T @ R:
    fft2(X)  = mm(mm(X, F), F)          (no transposes needed, F^T = F)
    ifft2(X) = mm(mm(X, conj(F)), conj(F))
Complex arithmetic is carried as separate real/imag planes; each complex
matmul is 4 real matmuls pair-accumulated in PSUM.

Data layout on chip: a 320x320 plane is stored as [128 partitions, 960] =
three row-tiles of 128/128/64 rows side by side in the free dim (rows padded
to 384 on the host so each plane loads in one DMA and pad regions are zero).
"""

import os

import numpy as np

B, C, H, W = 8, 16, 320, 320
NUM_ITER = 10
_DBG_ITERS = int(os.environ.get("KDBG_ITERS", NUM_ITER))
_DBG_COILS = int(os.environ.get("KDBG_COILS", C))
# NOTE: the For_i repeat-loop benchmarking path (reps > 1) crashes the
# execution unit on this runtime (NRT_EXEC_UNIT_UNRECOVERABLE) — keep it
# permanently disabled; the single-shot path below is the validated one.
_REPS = 1
RT = (128, 128, 64)          # row-tile sizes (320 = 128 + 128 + 64)
PLANE = 960                  # free-dim footprint of one plane
# full-width region plus the two garbage-safe regions for Z-derived data
REGIONS = ((0, 128, 0, 640), (0, 64, 640, 320))   # (p0, np, f0, nf)

_CACHE = {}


def _dft_mats():
    jj = np.arange(H)
    Wm = np.exp(-2j * np.pi * np.outer(jj, jj) / H) / np.sqrt(H)
    Fr = Wm.real.astype(np.float32)
    Fi = Wm.imag.astype(np.float32)
    return Fr, Fi


def _build():
    import concourse.mybir as mybir
    import concourse.tile as tile
    from concourse import bacc

    F32 = mybir.dt.float32
    F32R = mybir.dt.float32r
    F16 = mybir.dt.float16
    MUL = mybir.AluOpType.mult
    ADD = mybir.AluOpType.add
    SUB = mybir.AluOpType.subtract

    nc = bacc.Bacc("TRN2", target_bir_lowering=False, debug=False, num_devices=8)

    rhs_d = nc.dram_tensor("rhs", [2, 384, 320], F32, kind="ExternalInput").ap()
    csm_d = nc.dram_tensor("csm", [C, 2, 384, 320], F16, kind="ExternalInput").ap()
    mask_d = nc.dram_tensor("mask", [384, 320], F32, kind="ExternalInput").ap()
    fmat_d = nc.dram_tensor("fmat", [3, 384, 320], F32R, kind="ExternalInput").ap()
    lam_d = nc.dram_tensor("lam", [128, 1], F32, kind="ExternalInput").ap()
    out_d = nc.dram_tensor("out", [2, 320, 320], F32, kind="ExternalOutput").ap()

    ve = nc.vector
    gp = nc.gpsimd
    sc = nc.scalar

    def tt(eng, dst, doff, a, aoff, b, boff, op, safe=False):
        regions = REGIONS if safe else ((0, 128, 0, 960),)
        for (p0, np_, f0, nf) in regions:
            eng.tensor_tensor(dst[p0:p0 + np_, doff + f0:doff + f0 + nf],
                              a[p0:p0 + np_, aoff + f0:aoff + f0 + nf],
                              b[p0:p0 + np_, boff + f0:boff + f0 + nf], op)

    def stt(eng, dst, doff, a, aoff, scal, b, boff):
        # dst = a * scal + b   over both planes' regions (doff is plane offset)
        for (p0, np_, f0, nf) in REGIONS:
            eng.scalar_tensor_tensor(dst[p0:p0 + np_, doff + f0:doff + f0 + nf],
                                     a[p0:p0 + np_, aoff + f0:aoff + f0 + nf],
                                     scal[p0:p0 + np_, 0:1],
                                     b[p0:p0 + np_, boff + f0:boff + f0 + nf],
                                     MUL, ADD)

    with tile.TileContext(nc) as tc:
        with tc.tile_pool(name="const", bufs=1) as cpool, \
             tc.tile_pool(name="state", bufs=1) as spool, \
             tc.tile_pool(name="work", bufs=4) as wpool, \
             tc.tile_pool(name="prod", bufs=8) as ppool, \
             tc.tile_pool(name="sml", bufs=24) as mpool, \
             tc.tile_pool(name="ps", bufs=6, space="PSUM") as pspool, \
             tc.tile_pool(name="pssml", bufs=2, space="PSUM") as pspool2:

            csm_t = cpool.tile([128, C * 2 * PLANE], F16, tag="csm")
            fmat_t = cpool.tile([128, 3 * PLANE], F32R, tag="fmat")
            mask_t = cpool.tile([128, PLANE], F32, tag="mask")
            ones_t = cpool.tile([128, 128], F32, tag="ones")
            lam_t = cpool.tile([128, 1], F32, tag="lam")

            rhs_t = spool.tile([128, 2 * PLANE], F32, tag="rhs")
            p_t = spool.tile([128, 2 * PLANE], F32, tag="p")
            r_t = spool.tile([128, 2 * PLANE], F32, tag="r")
            x_t = spool.tile([128, 2 * PLANE], F32, tag="x")
            ap_t = spool.tile([128, 2 * PLANE], F32, tag="ap")

            # ---- input DMAs ----
            gp.dma_start(
                csm_t[:].rearrange("p (ctwo rt w) -> p ctwo rt w", rt=3, w=320),
                csm_d.rearrange("c two (rt p) w -> p (c two) rt w", p=128))
            gp.dma_start(
                fmat_t[:].rearrange("p (f rt w) -> p f rt w", rt=3, w=320),
                fmat_d.rearrange("f (rt p) w -> p f rt w", p=128))
            gp.dma_start(
                mask_t[:].rearrange("p (rt w) -> p rt w", w=320),
                mask_d.rearrange("(rt p) w -> p rt w", p=128))
            gp.dma_start(
                rhs_t[:].rearrange("p (two rt w) -> p two rt w", rt=3, w=320),
                rhs_d.rearrange("two (rt p) w -> p two rt w", p=128))
            gp.dma_start(lam_t[:], lam_d)
            ve.memset(ones_t[:], 1.0)

            def reduce_pair(a, aoff, b, boff):
                """sum over both planes of a[plane]*b[plane] -> PSUM [128,1]
                (same total in every partition)."""
                scr1 = ppool.tile([128, PLANE], F32, tag="prod")
                scr2 = ppool.tile([128, PLANE], F32, tag="prod")
                sA = mpool.tile([128, 1], F32, tag="sml")
                sB = mpool.tile([128, 1], F32, tag="sml")
                sC = mpool.tile([128, 1], F32, tag="sml")
                sD = mpool.tile([128, 1], F32, tag="sml")
                sAB = mpool.tile([128, 1], F32, tag="sml")
                sCD = mpool.tile([128, 1], F32, tag="sml")
                for scr, poff in ((scr1, 0), (scr2, PLANE)):
                    ve.tensor_tensor(scr[:, 0:640], a[:, aoff + poff:aoff + poff + 640],
                                     b[:, boff + poff:boff + poff + 640], MUL)
                    ve.tensor_tensor(scr[0:64, 640:960],
                                     a[0:64, aoff + poff + 640:aoff + poff + 960],
                                     b[0:64, boff + poff + 640:boff + poff + 960], MUL)
                ve.reduce_sum(sA[:], scr1[:, 0:640], axis=mybir.AxisListType.X)
                ve.reduce_sum(sB[:], scr2[:, 0:640], axis=mybir.AxisListType.X)
                ve.reduce_sum(sC[0:64, :], scr1[0:64, 640:960], axis=mybir.AxisListType.X)
                ve.reduce_sum(sD[0:64, :], scr2[0:64, 640:960], axis=mybir.AxisListType.X)
                ve.tensor_tensor(sAB[:], sA[:], sB[:], ADD)
                ve.tensor_tensor(sCD[0:64, :], sC[0:64, :], sD[0:64, :], ADD)
                tp = pspool2.tile([128, 1], F32, tag="pssml")
                nc.tensor.matmul(tp[:], ones_t[:, :], sAB[:], start=True, stop=False)
                nc.tensor.matmul(tp[:], ones_t[0:64, :], sCD[0:64, :], start=False, stop=True)
                return tp

            import contextlib
            rep_ctx = tc.For_i(0, _REPS, 1) if _REPS > 1 else contextlib.nullcontext()
            rep_stack = contextlib.ExitStack()
            rep_stack.enter_context(rep_ctx)
            sc.copy(p_t[:], rhs_t[:])
            sc.copy(r_t[:], rhs_t[:])
            ve.memset(x_t[:], 0.0)
            # initial rTr (r == rhs)
            rtr_ps = reduce_pair(r_t, 0, r_t, 0)
            rtr_sb = mpool.tile([128, 1], F32, tag="sml")
            rtr_rcp = mpool.tile([128, 1], F32, tag="sml")
            ve.tensor_copy(rtr_sb[:], rtr_ps[:])
            ve.reciprocal(rtr_rcp[:], rtr_ps[:])

            # stage term tables: list of (x_plane_off, f_block) per output plane
            FFT_R = ((0, 0), (PLANE, 2))   # Xr*Fr + Xi*(-Fi)
            FFT_I = ((0, 1), (PLANE, 0))   # Xr*Fi + Xi*Fr
            IFT_R = ((0, 0), (PLANE, 1))   # Xr*Fr + Xi*Fi
            IFT_I = ((PLANE, 0), (0, 2))   # Xi*Fr + Xr*(-Fi)

            def stage(x_tile, terms_r, terms_i, evac):
                for mt in range(3):
                    m = RT[mt]
                    for plane, terms in ((0, terms_r), (1, terms_i)):
                        pt = pspool.tile([128, 320], F32, tag="ps")
                        i = 0
                        for (xoff, fb) in terms:
                            for kt in range(3):
                                k = RT[kt]
                                nc.tensor.matmul(
                                    pt[0:m, :],
                                    x_tile[0:k, xoff + kt * 320 + mt * 128:
                                           xoff + kt * 320 + mt * 128 + m],
                                    fmat_t[0:k, fb * PLANE + kt * 320:
                                           fb * PLANE + (kt + 1) * 320],
                                    start=(i == 0), stop=(i == 5))
                                i += 1
                        evac(pt, mt, m, plane)

            for it in range(_DBG_ITERS):
                # Ap := lam * p   (coil contributions accumulate on top)
                for plane in (0, 1):
                    for (p0, np_, f0, nf) in REGIONS:
                        sc.activation(ap_t[p0:p0 + np_, plane * PLANE + f0:plane * PLANE + f0 + nf],
                                      p_t[p0:p0 + np_, plane * PLANE + f0:plane * PLANE + f0 + nf],
                                      mybir.ActivationFunctionType.Copy,
                                      scale=lam_t[p0:p0 + np_, 0:1])

                for c in range(_DBG_COILS):
                    so_r = (2 * c) * PLANE
                    so_i = (2 * c + 1) * PLANE
                    # ---- forward: G = csm_c * p (complex) ----
                    ma = ppool.tile([128, PLANE], F32, tag="prod")
                    mb = ppool.tile([128, PLANE], F32, tag="prod")
                    mc_ = ppool.tile([128, PLANE], F32, tag="prod")
                    md = ppool.tile([128, PLANE], F32, tag="prod")
                    tt(gp, ma, 0, csm_t, so_r, p_t, 0, MUL)          # Sr*pr
                    tt(gp, mb, 0, csm_t, so_i, p_t, PLANE, MUL)      # Si*pi
                    tt(ve, mc_, 0, csm_t, so_r, p_t, PLANE, MUL)     # Sr*pi
                    tt(ve, md, 0, csm_t, so_i, p_t, 0, MUL)          # Si*pr
                    g_t = wpool.tile([128, 2 * PLANE], F32R, tag="work")
                    tt(ve, g_t, 0, ma, 0, mb, 0, SUB)                # Gr
                    tt(ve, g_t, PLANE, mc_, 0, md, 0, ADD)           # Gi

                    # ---- fft stage 1 ----
                    b_t = wpool.tile([128, 2 * PLANE], F32R, tag="work")

                    def evac_copy(dst):
                        def f(pt, mt, m, plane):
                            sc.copy(dst[0:m, plane * PLANE + mt * 320:
                                        plane * PLANE + mt * 320 + 320], pt[0:m, :])
                        return f

                    stage(g_t, FFT_R, FFT_I, evac_copy(b_t))

                    # ---- fft stage 2 + mask ----
                    k_t = wpool.tile([128, 2 * PLANE], F32R, tag="work")

                    def evac_mask(pt, mt, m, plane):
                        ve.tensor_tensor(k_t[0:m, plane * PLANE + mt * 320:
                                             plane * PLANE + mt * 320 + 320],
                                         pt[0:m, :],
                                         mask_t[0:m, mt * 320:mt * 320 + 320], MUL)

                    stage(b_t, FFT_R, FFT_I, evac_mask)

                    # ---- ifft stage 1 ----
                    c_t = wpool.tile([128, 2 * PLANE], F32R, tag="work")
                    stage(k_t, IFT_R, IFT_I, evac_copy(c_t))

                    # ---- ifft stage 2 ----
                    zr = ppool.tile([128, PLANE], F32, tag="prod")
                    zi = ppool.tile([128, PLANE], F32, tag="prod")

                    def evac_z(pt, mt, m, plane):
                        dst = zr if plane == 0 else zi
                        sc.copy(dst[0:m, mt * 320:mt * 320 + 320], pt[0:m, :])

                    stage(c_t, IFT_R, IFT_I, evac_z)

                    # ---- backward: Ap += conj(csm_c) * Z ----
                    t1 = ppool.tile([128, PLANE], F32, tag="prod")
                    t2 = ppool.tile([128, PLANE], F32, tag="prod")
                    t3 = ppool.tile([128, PLANE], F32, tag="prod")
                    t4 = ppool.tile([128, PLANE], F32, tag="prod")
                    tt(gp, t1, 0, csm_t, so_r, zr, 0, MUL, safe=True)   # Sr*Zr
                    tt(gp, t2, 0, csm_t, so_i, zi, 0, MUL, safe=True)   # Si*Zi
                    tt(ve, t3, 0, csm_t, so_r, zi, 0, MUL, safe=True)   # Sr*Zi
                    tt(ve, t4, 0, csm_t, so_i, zr, 0, MUL, safe=True)   # Si*Zr
                    u = ppool.tile([128, PLANE], F32, tag="prod")
                    v = ppool.tile([128, PLANE], F32, tag="prod")
                    tt(ve, u, 0, t1, 0, t2, 0, ADD, safe=True)
                    tt(ve, v, 0, t3, 0, t4, 0, SUB, safe=True)
                    tt(ve, ap_t, 0, ap_t, 0, u, 0, ADD, safe=True)
                    tt(ve, ap_t, PLANE, ap_t, PLANE, v, 0, ADD, safe=True)

                # ---- CG scalar updates ----
                pap_ps = reduce_pair(p_t, 0, ap_t, 0)
                pap_rcp = mpool.tile([128, 1], F32, tag="sml")
                ve.reciprocal(pap_rcp[:], pap_ps[:])
                alpha = mpool.tile([128, 1], F32, tag="sml")
                nalpha = mpool.tile([128, 1], F32, tag="sml")
                ve.tensor_tensor(alpha[:], rtr_sb[:], pap_rcp[:], MUL)
                ve.tensor_scalar_mul(nalpha[:], alpha[:], -1.0)

                # x += alpha * p (off critical path); r -= alpha * Ap
                for plane_off in (0, PLANE):
                    stt(ve, x_t, plane_off, p_t, plane_off, alpha, x_t, plane_off)
                    stt(ve, r_t, plane_off, ap_t, plane_off, nalpha, r_t, plane_off)

                rtrn_ps = reduce_pair(r_t, 0, r_t, 0)
                rtrn_sb = mpool.tile([128, 1], F32, tag="sml")
                beta = mpool.tile([128, 1], F32, tag="sml")
                ve.tensor_copy(rtrn_sb[:], rtrn_ps[:])
                ve.tensor_tensor(beta[:], rtrn_sb[:], rtr_rcp[:], MUL)
                if it < _DBG_ITERS - 1:
                    rtr_rcp = mpool.tile([128, 1], F32, tag="sml")
                    ve.reciprocal(rtr_rcp[:], rtrn_ps[:])
                rtr_sb = rtrn_sb

                # p = beta * p + r
                for plane_off in (0, PLANE):
                    stt(ve, p_t, plane_off, p_t, plane_off, beta, r_t, plane_off)

            rep_stack.close()

            # ---- output ----
            for plane in (0, 1):
                for rt in range(3):
                    r = RT[rt]
                    gp.dma_start(out_d[plane, rt * 128:rt * 128 + r, :],
                                 x_t[0:r, plane * PLANE + rt * 320:
                                     plane * PLANE + rt * 320 + 320])

    nc.compile()
    return nc


def _get_nc():
    key = ("nc", _DBG_ITERS, _DBG_COILS, _REPS)
    if key not in _CACHE:
        _CACHE[key] = _build()
    return _CACHE[key]


def kernel(rhs, csm, mask, lam):
    from concourse.bass_utils import run_bass_kernel_spmd

    nc = _get_nc()

    Fr, Fi = _dft_mats()
    fmat = np.zeros((3, 384, 320), np.float32)
    fmat[0, :320] = Fr
    fmat[1, :320] = Fi
    fmat[2, :320] = -Fi
    lam_b = np.full((128, 1), np.float32(lam[0]), np.float32)

    in_maps = []
    for b in range(B):
        rhs_p = np.zeros((2, 384, 320), np.float32)
        rhs_p[:, :320] = rhs[b]
        csm_p = np.zeros((C, 2, 384, 320), np.float16)
        csm_p[:, :, :320] = np.moveaxis(csm[b], -1, 1).astype(np.float16)
        mask_p = np.zeros((384, 320), np.float32)
        mask_p[:320] = mask[b].astype(np.float32)
        in_maps.append({"rhs": rhs_p, "csm": csm_p, "mask": mask_p,
                        "fmat": fmat, "lam": lam_b})

    trace = bool(int(os.environ.get("KBENCH_TRACE", "0")))
    res = run_bass_kernel_spmd(nc, in_maps, core_ids=list(range(8)), trace=trace)
    _CACHE["last_result"] = res

    out = np.empty((B, H, W, 2), np.float32)
    for b in range(B):
        o = res.results[b]["out"]
        out[b, :, :, 0] = o[0]
        out[b, :, :, 1] = o[1]
    return out

